# revision 9
# baseline (speedup 1.0000x reference)
"""Trainium2 Bass kernel for nn_LucaGPLMMultiheadAttention.

MHA with RoPE: S=2048, B=2, E=1024, H=16, hd=64, fp32.
Sharding: head-parallel across 8 cores (2 heads x 2 batch = 4 (b,h) pairs
per core). q/k/v projections column-split, out projection row-split with an
on-device ReduceScatter over the sequence axis; host concatenates shards and
adds the output bias (bo) once.

All big matmuls run as float32r (fp32 streamed at full rate when the moving
free dim >= 256; TF32-like rounding, ~3e-4 rel err per matmul).

Schedule: the program is emitted as interleaved chunks so engines overlap:
  ph1(b0,s0..s3) qb0(b0) s0(b1) op(qb0) s1(b1) qb1(b0) s2(b1) op(qb1)
  s3(b1) qb0(b1) op qb1(b1) op
Each engine executes its instructions in program order, so batch-1
projections/rope (PE/DVE/Pool) run underneath batch-0's exp-paced attention
(ACT). One PSUM pool fills all 8 banks: a shared [128,512] 'u' tag (bufs=2)
serves transposes, projections and out-projection; 'sc' [128,1024] (bufs=2)
pipelines scores vs exp; 'at' [65,1024] (bufs=1) accumulates attn + the
ones-column row sums.

Engine budget per core (cost model): PE ~190us (floor: scores+attn are
output/moving-lane-bound at 128 lanes/cycle), ACT ~134us (exp only -- all
copies live on the idle GPSIMD/Pool engine), DVE ~105us, Pool ~115us.
"""

import os
import sys

sys.path.insert(0, "/opt/trn_rl_repo")

import numpy as np

S = 2048
B = 2
E = 1024
H = 16
HD = 64
NCORES = 8
HPC = H // NCORES  # heads per core = 2
EL = HPC * HD  # local embed slice = 128
SB = S * B  # 4096 rows
SHARD = SB // NCORES  # 512 rows per core after reduce-scatter
QB = 1024  # qi block size

_CACHE: dict = {}
LAST_RESULT = None


def _build_program(with_cc: bool = True):
    import concourse.mybir as mybir
    import concourse.tile as tile
    from concourse import bacc
    from concourse.masks import make_identity

    f32 = mybir.dt.float32
    f32r = mybir.dt.float32r
    Exp = mybir.ActivationFunctionType.Exp
    add = mybir.AluOpType.add
    mult = mybir.AluOpType.mult

    nc = bacc.Bacc(
        "TRN2",
        target_bir_lowering=False,
        debug=False,
        enable_asserts=False,
        num_devices=NCORES,
    )

    def din(name, shape):
        return nc.dram_tensor(name, shape, f32, kind="ExternalInput").ap()

    query = din("query", [S, B, E])
    q_w = din("q_w", [E, EL])  # (Wq_slice * scaling).T
    k_w = din("k_w", [E, EL])
    v_w = din("v_w", [E, EL])
    o_w = din("o_w", [EL, E])  # Wo[:, slice].T
    bq_s = din("bq_s", [EL, 1])
    bk_s = din("bk_s", [EL, 1])
    bv_s = din("bv_s", [EL, 1])
    cos_t = din("cos_t", [EL, S])  # 2-head stacked rope tables (sin sign-folded)
    sin_t = din("sin_t", [EL, S])
    out_ext = nc.dram_tensor("out", [SHARD, E], f32, kind="ExternalOutput").ap()

    with tile.TileContext(nc) as tc:
        with (
            tc.tile_pool(name="const", bufs=1) as const,
            tc.tile_pool(name="persist", bufs=1) as persist,
            tc.tile_pool(name="dram", bufs=1, space="DRAM") as dram,
            # one PSUM pool, 16KB/partition exactly:
            #   u  [128,512] f32 x2 = 4KB  (transposes, projections, out-proj)
            #   sc [128,1024] f32 x2 = 8KB (scores / exp pipeline)
            #   at [65,1024] f32 x1 = 4KB  (attn accumulation + row sums)
            tc.tile_pool(name="ps", bufs=1, space="PSUM") as ps_pool,
            tc.tile_pool(name="ld", bufs=2) as ld,
            tc.tile_pool(name="qtb", bufs=2) as qtb,
            tc.tile_pool(name="vblk", bufs=2) as vblk_pool,
            tc.tile_pool(name="rope", bufs=2) as rope,
            tc.tile_pool(name="probs", bufs=3) as probs_pool,
            tc.tile_pool(name="attn_sb", bufs=2) as attn_sb,
            tc.tile_pool(name="osb", bufs=2) as osb,
        ):
            # ---- constants to SBUF (weights staged fp32, rounded to f32r) ----
            qw_sb = const.tile([128, 8, EL], f32r, name="qw_sb")
            kw_sb = const.tile([128, 8, EL], f32r, name="kw_sb")
            vw_sb = const.tile([128, 8, EL], f32r, name="vw_sb")
            ow_sb = const.tile([EL, E], f32r, name="ow_sb")
            bq_sb = const.tile([EL, 1], f32, name="bq_sb")
            bk_sb = const.tile([EL, 1], f32, name="bk_sb")
            bv_sb = const.tile([EL, 1], f32, name="bv_sb")
            ident = const.tile([128, 128], f32, name="ident")
            identr = const.tile([128, 128], f32r, name="identr")
            # two I_64 stacked on partitions 0:64 and 64:128 (for h=1 transposes)
            id64r = const.tile([128, HD], f32r, name="id64r")
            cos_sb = const.tile([EL, S], f32, name="cos_sb")
            sin_sb = const.tile([EL, S], f32, name="sin_sb")

            with tc.tile_pool(name="wstage", bufs=1) as wstage:
                for src, dst in ((q_w, qw_sb), (k_w, kw_sb), (v_w, vw_sb)):
                    stg = wstage.tile([128, 8, EL], f32, tag="wstg")
                    nc.sync.dma_start(stg[:], src.rearrange("(c p) m -> p c m", p=128))
                    nc.vector.tensor_copy(dst[:], stg[:])
                stg = wstage.tile([EL, E], f32, tag="owstg")
                nc.sync.dma_start(stg[:], o_w[:])
                nc.vector.tensor_copy(ow_sb[:], stg[:])

            nc.sync.dma_start(bq_sb[:], bq_s[:])
            nc.sync.dma_start(bk_sb[:], bk_s[:])
            nc.sync.dma_start(bv_sb[:], bv_s[:])
            nc.sync.dma_start(cos_sb[:], cos_t[:])
            nc.sync.dma_start(sin_sb[:], sin_t[:])
            make_identity(nc, ident[:])
            nc.vector.tensor_copy(identr[:], ident[:])
            nc.vector.tensor_copy(id64r[0:HD, :], ident[0:HD, 0:HD])
            nc.vector.tensor_copy(id64r[HD:128, :], ident[0:HD, 0:HD])

            # ---- persistent activations ----
            qT = persist.tile([EL, SB], f32r, name="qT")  # [2h*hd, b-major cols]
            kT = persist.tile([EL, SB], f32r, name="kT")
            # v kj-tiles [128, 64] + ones column, built in ph1
            vaug = persist.tile([128, HPC * B * 16, HD + 1], f32r, name="vaug")
            ones_f = const.tile([128, HPC * B * 16], f32, name="ones_f")
            nc.vector.memset(ones_f[:], 1.0)
            nc.vector.tensor_copy(vaug[:, :, HD], ones_f[:])

            P_dram = [dram.tile([S, E], f32, name=f"P_dram{b}") for b in range(B)]
            rs_out = [
                dram.tile([S // NCORES, E], f32, name=f"rs_out{b}") for b in range(B)
            ]

            # ---- phase-1 chunk: transpose query, project, rope, v-tiles ----
            def ph1_block(b, sblk):
                col0 = b * S + sblk * 512
                qt_blk = qtb.tile([128, 8, 512], f32r, tag="qt_blk")
                v_blk = vblk_pool.tile([128, 512], f32r, tag="v_blk")
                halves = []
                for hf in range(2):
                    qry = ld.tile([128, 2, E], f32, tag="qry")
                    nc.sync.dma_start(
                        qry[:],
                        query[
                            sblk * 512 + hf * 256 : sblk * 512 + (hf + 1) * 256, b
                        ].rearrange("(i p) e -> p i e", p=128),
                    )
                    halves.append(qry)
                for i in range(4):
                    qry = halves[i // 2]
                    ii = i % 2
                    # 4 transposes share one 1-bank psum tile (fp32: the
                    # verifier requires f32r matmult operands to come from a
                    # rounding instruction, so the raw DMA'd query stays fp32)
                    for eg in range(2):
                        tp = ps_pool.tile([128, 512], f32, tag="u")
                        for ec2 in range(4):
                            ec = eg * 4 + ec2
                            nc.tensor.transpose(
                                tp[:, ec2 * 128 : (ec2 + 1) * 128],
                                qry[:, ii, ec * 128 : (ec + 1) * 128],
                                ident[:],
                            )
                        nc.vector.tensor_copy(
                            qt_blk[
                                :, eg * 4 : (eg + 1) * 4, i * 128 : (i + 1) * 128
                            ],
                            tp[:].rearrange("p (c m) -> p c m", c=4),
                        )
                for w_sb, bias, dst, do_rope in (
                    (qw_sb, bq_sb, qT[:, col0 : col0 + 512], True),
                    (kw_sb, bk_sb, kT[:, col0 : col0 + 512], True),
                    (vw_sb, bv_sb, v_blk[:], False),
                ):
                    pj = ps_pool.tile([128, 512], f32, tag="u")
                    for ec in range(8):
                        nc.tensor.matmul(
                            pj[:],
                            w_sb[:, ec, :],
                            qt_blk[:, ec, :],
                            start=(ec == 0),
                            stop=(ec == 7),
                        )
                    nc.vector.tensor_scalar_add(dst, pj[:], bias[:])
                    if do_rope:
                        # rope: x' = x*cos + shuffle(x)*sin_f (sign folded in
                        # sin). Shuffle via partition-shifted copies (walrus
                        # requires TensorTensor operands to share a start
                        # partition; only copies may shift).
                        ccol = slice(sblk * 512, (sblk + 1) * 512)
                        t1 = rope.tile([EL, 512], f32, tag="t1")
                        t2 = rope.tile([EL, 512], f32, tag="t2")
                        for h in range(HPC):
                            p0 = h * HD
                            nc.gpsimd.tensor_copy(
                                t2[p0 : p0 + 32, :], dst[p0 + 32 : p0 + 64, :]
                            )
                            nc.gpsimd.tensor_copy(
                                t2[p0 + 32 : p0 + 64, :], dst[p0 : p0 + 32, :]
                            )
                        nc.gpsimd.tensor_tensor(
                            out=t1[:], in0=dst, in1=cos_sb[:, ccol], op=mult
                        )
                        nc.gpsimd.tensor_tensor(
                            out=t2[:], in0=t2[:], in1=sin_sb[:, ccol], op=mult
                        )
                        nc.vector.tensor_tensor(
                            out=dst, in0=t1[:], in1=t2[:], op=add
                        )
                    else:
                        # v natural kj-tiles for this block
                        for kt2 in range(4):
                            kt = sblk * 4 + kt2
                            for h in range(HPC):
                                vt = ps_pool.tile([128, 512], f32, tag="u")
                                vtr = vt[:, 0:HD].bitcast(f32r)
                                nc.tensor.transpose(
                                    vtr,
                                    v_blk[
                                        h * HD : (h + 1) * HD,
                                        kt2 * 128 : (kt2 + 1) * 128,
                                    ],
                                    id64r[h * HD : (h + 1) * HD, :],
                                )
                                nc.vector.tensor_copy(
                                    vaug[:, (h * B + b) * 16 + kt, :HD], vtr
                                )

            # ---- attention chunk: one qi block (both heads), normalized ----
            def attn_chunk(b, qb):
                q0 = b * S + qb * QB
                attnT = attn_sb.tile([EL, QB], f32r, tag="attnT")
                for h in range(HPC):
                    hs = slice(h * HD, (h + 1) * HD)
                    pair = h * B + b
                    at = ps_pool.tile([HD + 1, QB], f32, tag="at")
                    for kt in range(16):
                        k0 = b * S + kt * 128
                        sc = ps_pool.tile([128, QB], f32, tag="sc")
                        for half in range(2):
                            nc.tensor.matmul(
                                sc[:, half * 512 : (half + 1) * 512],
                                kT[hs, k0 : k0 + 128],
                                qT[hs, q0 + half * 512 : q0 + (half + 1) * 512],
                                start=True,
                                stop=True,
                                skip_group_check=True,
                            )
                        pr = probs_pool.tile([128, QB], f32r, tag="pr")
                        nc.scalar.activation(pr[:], sc[:], Exp)
                        for half in range(2):
                            nc.tensor.matmul(
                                at[:, half * 512 : (half + 1) * 512],
                                vaug[:, pair * 16 + kt, :],
                                pr[:, half * 512 : (half + 1) * 512],
                                start=(kt == 0),
                                stop=(kt == 15),
                                skip_group_check=True,
                            )
                    # softmax denominators came along in the ones column.
                    # Broadcast recip to all 128 partitions so each head's
                    # normalize uses SBUF operands with matching start
                    # partition (attnT[hs] vs rbc[hs]).
                    srow = osb.tile([1, QB], f32, tag="srow")
                    rbc = osb.tile([128, QB], f32, tag="rbc")
                    nc.vector.reciprocal(srow[:], at[HD : HD + 1, :])
                    nc.gpsimd.partition_broadcast(rbc[:], srow[:])
                    nc.vector.tensor_tensor(
                        out=attnT[hs, :], in0=at[0:HD, :], in1=rbc[hs, :], op=mult
                    )
                return attnT

            def outproj(b, qb, attnT):
                for st2 in range(QB // 128):
                    st = qb * (QB // 128) + st2
                    for nch in range(2):
                        op = ps_pool.tile([128, 512], f32, tag="u")
                        nc.tensor.matmul(
                            op[:],
                            attnT[:, st2 * 128 : (st2 + 1) * 128],
                            ow_sb[:, nch * 512 : (nch + 1) * 512],
                            start=True,
                            stop=True,
                            skip_group_check=True,
                        )
                        psb = osb.tile([128, 512], f32, tag="ptile")
                        nc.vector.tensor_copy(psb[:], op[:])
                        nc.sync.dma_start(
                            P_dram[b][
                                st * 128 : (st + 1) * 128,
                                nch * 512 : (nch + 1) * 512,
                            ],
                            psb[:],
                        )

            def reduce_scatter(b):
                out_v = out_ext.rearrange("(s b) e -> s b e", b=B)
                if with_cc:
                    nc.gpsimd.collective_compute(
                        "ReduceScatter",
                        add,
                        replica_groups=[list(range(NCORES))],
                        ins=[P_dram[b].opt()],
                        outs=[rs_out[b].opt()],
                    )
                else:  # timeline-sim variant: no collective, copy shard 0
                    nc.sync.dma_start(rs_out[b][:], P_dram[b][0 : S // NCORES, :])
                nc.sync.dma_start(out_v[:, b, :], rs_out[b][:])

            # ---- interleaved schedule ----
            for sblk in range(4):
                ph1_block(0, sblk)
            aT = attn_chunk(0, 0)
            ph1_block(1, 0)
            outproj(0, 0, aT)
            ph1_block(1, 1)
            aT = attn_chunk(0, 1)
            ph1_block(1, 2)
            outproj(0, 1, aT)
            ph1_block(1, 3)
            reduce_scatter(0)
            aT = attn_chunk(1, 0)
            outproj(1, 0, aT)
            aT = attn_chunk(1, 1)
            outproj(1, 1, aT)
            reduce_scatter(1)

    nc.compile()
    return nc


def _host_inputs(query, Wq, bq, Wk, bk, Wv, bv, Wo, bo):
    """Per-core input maps (all fp32, C-contiguous)."""
    scaling = HD ** (-0.5)

    invf = 1.0 / (
        10000.0 ** (np.arange(0, HD, 2, dtype=np.float32) / np.float32(HD))
    )
    t = np.arange(S, dtype=np.float32)
    fr = np.outer(t, invf).astype(np.float32)  # [S, 32]
    emb = np.concatenate([fr, fr], axis=1)  # [S, HD]
    cosT = np.cos(emb).T.astype(np.float32)  # [HD, S]
    sinT = np.sin(emb).T.astype(np.float32)
    sign = np.where(np.arange(HD) < HD // 2, -1.0, 1.0).astype(np.float32)[:, None]
    cos_t = np.ascontiguousarray(np.tile(cosT, (HPC, 1)), dtype=np.float32)
    sin_t = np.ascontiguousarray(np.tile(sinT * sign, (HPC, 1)), dtype=np.float32)

    query = np.ascontiguousarray(query, dtype=np.float32)
    in_maps = []
    for c in range(NCORES):
        sl = slice(c * EL, (c + 1) * EL)
        in_maps.append(
            {
                "query": query,
                "q_w": np.ascontiguousarray((Wq[sl, :] * scaling).T, dtype=np.float32),
                "k_w": np.ascontiguousarray(Wk[sl, :].T, dtype=np.float32),
                "v_w": np.ascontiguousarray(Wv[sl, :].T, dtype=np.float32),
                "o_w": np.ascontiguousarray(Wo[:, sl].T, dtype=np.float32),
                "bq_s": np.ascontiguousarray(
                    (bq[sl] * scaling).reshape(EL, 1), dtype=np.float32
                ),
                "bk_s": np.ascontiguousarray(bk[sl].reshape(EL, 1), dtype=np.float32),
                "bv_s": np.ascontiguousarray(bv[sl].reshape(EL, 1), dtype=np.float32),
                "cos_t": cos_t,
                "sin_t": sin_t,
            }
        )
    return in_maps


def kernel(query, Wq, bq, Wk, bk, Wv, bv, Wo, bo):
    global LAST_RESULT
    from concourse.bass_utils import run_bass_kernel_spmd

    if "nc" not in _CACHE:
        _CACHE["nc"] = _build_program()
    nc = _CACHE["nc"]

    in_maps = _host_inputs(
        np.asarray(query),
        np.asarray(Wq),
        np.asarray(bq),
        np.asarray(Wk),
        np.asarray(bk),
        np.asarray(Wv),
        np.asarray(bv),
        np.asarray(Wo),
        np.asarray(bo),
    )
    res = run_bass_kernel_spmd(nc, in_maps, core_ids=list(range(NCORES)))
    LAST_RESULT = res
    shards = [
        res.results[c]["out"].reshape(S // NCORES, B, E) for c in range(NCORES)
    ]
    full = np.concatenate(shards, axis=0)
    # output bias is added once on the host (not per-core: the ReduceScatter
    # sums partial projections from all 8 cores)
    return full + np.asarray(bo, dtype=np.float32).reshape(1, 1, E)


# revision 49
# speedup vs baseline: 1.8729x; 1.8729x over previous
"""Trainium2 Bass kernel for nn_LucaGPLMMultiheadAttention.

MHA with RoPE: S=2048, B=2, E=1024, H=16, hd=64, fp32.
Sharding: head-parallel across 8 cores (2 heads x 2 batch = 4 (b,h) pairs
per core). q/k/v projections column-split, out projection row-split with an
on-device ReduceScatter over the sequence axis; host concatenates shards and
adds the output bias (bo) once.

All big matmuls run as float32r (fp32 streamed at full rate when the moving
free dim >= 256; TF32-like rounding, ~3e-4 rel err per matmul).

Schedule: the program is emitted as interleaved chunks so engines overlap:
  ph1(b0,s0..s3) qb0(b0) s0(b1) op(qb0) s1(b1) qb1(b0) s2(b1) op(qb1)
  s3(b1) qb0(b1) op qb1(b1) op
Each engine executes its instructions in program order, so batch-1
projections/rope (PE/DVE/Pool) run underneath batch-0's exp-paced attention
(ACT). One PSUM pool fills all 8 banks: a shared [128,512] 'u' tag (bufs=2)
serves transposes, projections and out-projection; 'sc' [128,1024] (bufs=2)
pipelines scores vs exp; 'at' [65,1024] (bufs=1) accumulates attn + the
ones-column row sums.

Engine budget per core (cost model): PE ~190us (floor: scores+attn are
output/moving-lane-bound at 128 lanes/cycle), ACT ~134us (exp only -- all
copies live on the idle GPSIMD/Pool engine), DVE ~105us, Pool ~115us.
"""

import os
import sys

sys.path.insert(0, "/opt/trn_rl_repo")

import numpy as np

S = 2048
B = 2
E = 1024
H = 16
HD = 64
NCORES = 8
HPC = H // NCORES  # heads per core = 2
EL = HPC * HD  # local embed slice = 128
SB = S * B  # 4096 rows
SHARD = SB // NCORES  # 512 rows per core after reduce-scatter
QB = 1024  # qi block size

_CACHE: dict = {}
LAST_RESULT = None


def _build_program(with_cc: bool = True):
    import concourse.mybir as mybir
    import concourse.tile as tile
    from concourse import bacc
    from concourse.masks import make_identity

    f32 = mybir.dt.float32
    f32r = mybir.dt.float32r
    Exp = mybir.ActivationFunctionType.Exp
    add = mybir.AluOpType.add
    mult = mybir.AluOpType.mult

    nc = bacc.Bacc(
        "TRN2",
        target_bir_lowering=False,
        debug=False,
        enable_asserts=False,
        num_devices=NCORES,
    )

    def din(name, shape):
        return nc.dram_tensor(name, shape, f32, kind="ExternalInput").ap()

    query = din("query", [S, B, E])
    q_w = din("q_w", [E, EL])  # (Wq_slice * scaling).T
    k_w = din("k_w", [E, EL])
    v_w = din("v_w", [E, EL])
    o_w = din("o_w", [EL, E])  # Wo[:, slice].T
    bq_s = din("bq_s", [EL, 1])
    bk_s = din("bk_s", [EL, 1])
    bv_s = din("bv_s", [EL, 1])
    cos_t = din("cos_t", [EL, S])  # 2-head stacked rope tables (sin sign-folded)
    sin_t = din("sin_t", [EL, S])
    # b-major shard layout: the reduce-scatter (or its no-collective stand-in)
    # writes its contiguous [S/NCORES, E] result straight into the external
    # output -- no final reshuffle DMA. The host reassembles shards.
    out_ext = nc.dram_tensor(
        "out", [B, S // NCORES, E], f32, kind="ExternalOutput"
    ).ap()

    with tile.TileContext(nc) as tc:
        with (
            tc.tile_pool(name="const", bufs=1) as const,
            tc.tile_pool(name="persist", bufs=1) as persist,
            tc.tile_pool(name="dram", bufs=1, space="DRAM") as dram,
            # one PSUM pool, 16KB/partition exactly:
            #   u  [128,512] f32 x2 = 4KB  (transposes, projections, out-proj)
            #   sc [128,1024] f32 x2 = 8KB (scores / exp pipeline)
            #   at [65,1024] f32 x1 = 4KB  (attn accumulation + row sums)
            tc.tile_pool(name="ps", bufs=1, space="PSUM") as ps_pool,
            tc.tile_pool(name="ld", bufs=4) as ld,
            tc.tile_pool(name="qtb", bufs=2) as qtb,
            tc.tile_pool(name="vblk", bufs=1) as vblk_pool,
            tc.tile_pool(name="rope", bufs=1) as rope,
            tc.tile_pool(name="probs", bufs=3) as probs_pool,
            tc.tile_pool(name="attn_sb", bufs=2) as attn_sb,
            tc.tile_pool(name="osb", bufs=2) as osb,
        ):
            # ---- constants to SBUF (weights staged fp32, rounded to f32r) ----
            qw_sb = const.tile([128, 8, EL], f32r, name="qw_sb")
            kw_sb = const.tile([128, 8, EL], f32r, name="kw_sb")
            vw_sb = const.tile([128, 8, EL], f32r, name="vw_sb")
            ow_sb = const.tile([EL, E], f32r, name="ow_sb")
            bq_sb = const.tile([EL, 1], f32, name="bq_sb")
            bk_sb = const.tile([EL, 1], f32, name="bk_sb")
            bv_sb = const.tile([EL, 1], f32, name="bv_sb")
            ident = const.tile([128, 128], f32, name="ident")
            identr = const.tile([128, 128], f32r, name="identr")
            # two I_64 stacked on partitions 0:64 and 64:128 (for h=1 transposes)
            id64r = const.tile([128, HD], f32r, name="id64r")
            cos_sb = const.tile([EL, S], f32, name="cos_sb")
            sin_sb = const.tile([EL, S], f32, name="sin_sb")

            def stage_weights():
                # emitted after the first query-block DMAs so the query (on
                # the critical path to the first transposes) transfers first;
                # o_w last (first needed only by the out-projection ~60us in)
                with tc.tile_pool(name="wstage", bufs=1) as wstage:
                    for src, dst in ((q_w, qw_sb), (k_w, kw_sb)):
                        stg = wstage.tile([128, 8, EL], f32, tag="wstg")
                        nc.sync.dma_start(
                            stg[:], src.rearrange("(c p) m -> p c m", p=128)
                        )
                        nc.vector.tensor_copy(dst[:], stg[:])
                    nc.sync.dma_start(bq_sb[:], bq_s[:])
                    nc.sync.dma_start(bk_sb[:], bk_s[:])
                    nc.sync.dma_start(cos_sb[:], cos_t[:])
                    nc.sync.dma_start(sin_sb[:], sin_t[:])
                    stg = wstage.tile([128, 8, EL], f32, tag="wstg")
                    nc.sync.dma_start(
                        stg[:], v_w.rearrange("(c p) m -> p c m", p=128)
                    )
                    nc.vector.tensor_copy(vw_sb[:], stg[:])
                    nc.sync.dma_start(bv_sb[:], bv_s[:])
                    stg = wstage.tile([EL, E], f32, tag="owstg")
                    nc.sync.dma_start(stg[:], o_w[:])
                    nc.vector.tensor_copy(ow_sb[:], stg[:])

            make_identity(nc, ident[:])
            nc.vector.tensor_copy(identr[:], ident[:])
            nc.vector.tensor_copy(id64r[0:HD, :], ident[0:HD, 0:HD])
            nc.vector.tensor_copy(id64r[HD:128, :], ident[0:HD, 0:HD])

            # ---- persistent activations ----
            qT = persist.tile([EL, SB], f32r, name="qT")  # [2h*hd, b-major cols]
            kT = persist.tile([EL, SB], f32r, name="kT")
            # v kj-tiles [128, 64] + ones column, built in ph1
            vaug = persist.tile([128, HPC * B * 16, HD + 1], f32r, name="vaug")
            ones_f = const.tile([128, HPC * B * 16], f32, name="ones_f")
            nc.vector.memset(ones_f[:], 1.0)
            nc.vector.tensor_copy(vaug[:, :, HD], ones_f[:])

            P_dram = [dram.tile([S, E], f32, name=f"P_dram{b}") for b in range(B)]
            rs_out = [
                dram.tile([S // NCORES, E], f32, name=f"rs_out{b}")
                for b in range(B)
            ]


            # ---- phase-1: transpose query, project, rope, v-tiles ----
            # Emitted as closures per 512-row block so the work can be
            # injected into attention chunks as PE fillers. Rope closures are
            # separated from projections and emitted last: the rope-add (DVE)
            # waits on Pool multiplies, and sitting in the middle of the DVE
            # FIFO it head-of-line blocks every later block's psum copies.
            # act_copies=True routes the psum->sbuf copies to the (otherwise
            # idle) ACT engine -- used in the prologue, before the first exp.
            def ph1_chunks(b, sblk, act_copies=False, rope_dve=False):
                col0 = b * S + sblk * 512
                state = {}

                def copy_engine(dst, src):
                    if act_copies:
                        nc.scalar.activation(
                            dst, src, mybir.ActivationFunctionType.Copy
                        )
                    else:
                        nc.vector.tensor_copy(dst, src)

                def c_start():
                    state["qt"] = qtb.tile(
                        [128, 8, 512], f32r, tag="qt_blk", name=f"qt_{b}_{sblk}"
                    )
                    state["v"] = vblk_pool.tile(
                        [128, 512], f32r, tag="v_blk", name=f"v_{b}_{sblk}"
                    )
                    state["halves"] = []
                    for hf in range(2):
                        qry = ld.tile([128, 2, E], f32, tag="qry")
                        nc.sync.dma_start(
                            qry[:],
                            query[
                                sblk * 512 + hf * 256 : sblk * 512
                                + (hf + 1) * 256,
                                b,
                            ].rearrange("(i p) e -> p i e", p=128),
                        )
                        state["halves"].append(qry)

                def c_tp(i, eg):
                    # fp32 transposes: the verifier requires f32r matmult
                    # operands to come from a rounding instruction, so the
                    # raw DMA'd query stays fp32
                    qry = state["halves"][i // 2]
                    ii = i % 2
                    qt_blk = state["qt"]
                    tp = ps_pool.tile([128, 512], f32, tag="u", bufs=2)
                    for ec2 in range(4):
                        ec = eg * 4 + ec2
                        nc.tensor.transpose(
                            tp[:, ec2 * 128 : (ec2 + 1) * 128],
                            qry[:, ii, ec * 128 : (ec + 1) * 128],
                            ident[:],
                        )
                    copy_engine(
                        qt_blk[
                            :, eg * 4 : (eg + 1) * 4, i * 128 : (i + 1) * 128
                        ],
                        tp[:].rearrange("p (c m) -> p c m", c=4),
                    )

                def c_proj(which):
                    w_sb, bias = (
                        (qw_sb, bq_sb),
                        (kw_sb, bk_sb),
                        (vw_sb, bv_sb),
                    )[which]
                    dst = (
                        qT[:, col0 : col0 + 512],
                        kT[:, col0 : col0 + 512],
                        state["v"][:],
                    )[which]
                    qt_blk = state["qt"]
                    pj = ps_pool.tile([128, 512], f32, tag="u", bufs=2)
                    for ec in range(8):
                        nc.tensor.matmul(
                            pj[:],
                            w_sb[:, ec, :],
                            qt_blk[:, ec, :],
                            start=(ec == 0),
                            stop=(ec == 7),
                        )
                    nc.vector.tensor_scalar_add(dst, pj[:], bias[:])
                    if which == 2:
                        # v natural kj-tiles for this block (f32r transpose:
                        # v_blk is DVE-rounded, so 1.5 cycles/row)
                        v_blk = state["v"]
                        for kt2 in range(4):
                            kt = sblk * 4 + kt2
                            for h in range(HPC):
                                vt = ps_pool.tile([128, 512], f32, tag="u", bufs=2)
                                vtr = vt[:, 0:HD].bitcast(f32r)
                                nc.tensor.transpose(
                                    vtr,
                                    v_blk[
                                        h * HD : (h + 1) * HD,
                                        kt2 * 128 : (kt2 + 1) * 128,
                                    ],
                                    id64r[h * HD : (h + 1) * HD, :],
                                )
                                copy_engine(
                                    vaug[:, (h * B + b) * 16 + kt, :HD], vtr
                                )

                def c_rope(which):
                    # rope: x' = x*cos + shuffle(x)*sin_f (sign folded in
                    # sin). Shuffle via partition-shifted copies (walrus
                    # requires TensorTensor operands to share a start
                    # partition; only copies may shift). rope_dve picks the
                    # engine lane so concurrent blocks' ropes don't queue
                    # behind each other.
                    eng = nc.vector if rope_dve else nc.gpsimd
                    dst = (qT, kT)[which][:, col0 : col0 + 512]
                    ccol = slice(sblk * 512, (sblk + 1) * 512)
                    t1 = rope.tile([EL, 512], f32, tag="t1")
                    t2 = rope.tile([EL, 512], f32, tag="t2")
                    for h in range(HPC):
                        p0 = h * HD
                        eng.tensor_copy(
                            t2[p0 : p0 + 32, :], dst[p0 + 32 : p0 + 64, :]
                        )
                        eng.tensor_copy(
                            t2[p0 + 32 : p0 + 64, :], dst[p0 : p0 + 32, :]
                        )
                    eng.tensor_tensor(
                        out=t1[:], in0=dst, in1=cos_sb[:, ccol], op=mult
                    )
                    eng.tensor_tensor(
                        out=t2[:], in0=t2[:], in1=sin_sb[:, ccol], op=mult
                    )
                    eng.tensor_tensor(out=dst, in0=t1[:], in1=t2[:], op=add)

                return {
                    "start": c_start,
                    "tps": [
                        lambda i=i, eg=eg: c_tp(i, eg)
                        for i in range(4)
                        for eg in range(2)
                    ],
                    "projq": lambda: c_proj(0),
                    "projk": lambda: c_proj(1),
                    "projv": lambda: c_proj(2),
                    "ropeq": lambda: c_rope(0),
                    "ropek": lambda: c_rope(1),
                }

            # ---- attention chunk: one qi block (both heads), normalized ----
            # Software-pipelined: scores(step+1) is emitted BEFORE the attn
            # matmuls of the current step, so the in-order PE queue never
            # blocks behind exp(step) -- the cadence becomes max(ACT, PE)
            # instead of their serial sum. The finished at-psum tile is
            # released by one immediate DVE copy; normalize runs off-psum.
            # `fillers` are PE-side work chunks (out-projection) injected
            # mid-loop to fill exp-paced PE idle without touching ACT.
            def attn_chunk(b, qb, fillers=(), atc_on_act=False):
                q0 = b * S + qb * QB
                attnT = attn_sb.tile([EL, QB], f32r, tag="attnT")
                steps = [(h, kt) for h in range(HPC) for kt in range(16)]

                def emit_sc(h, kt):
                    hs = slice(h * HD, (h + 1) * HD)
                    k0 = b * S + kt * 128
                    sc = ps_pool.tile([128, QB], f32, tag="sc", bufs=2)
                    for half in range(2):
                        nc.tensor.matmul(
                            sc[:, half * 512 : (half + 1) * 512],
                            kT[hs, k0 : k0 + 128],
                            qT[hs, q0 + half * 512 : q0 + (half + 1) * 512],
                            start=True,
                            stop=True,
                            skip_group_check=True,
                        )
                    return sc

                fill_iter = iter(fillers)
                at_tiles = [None, None]
                pr_tiles = {}

                def emit_at(idx):
                    h, kt = steps[idx]
                    pair = h * B + b
                    at = at_tiles[h]
                    pr = pr_tiles.pop(idx)
                    for half in range(2):
                        nc.tensor.matmul(
                            at[:, half * 512 : (half + 1) * 512],
                            vaug[:, pair * 16 + kt, :],
                            pr[:, half * 512 : (half + 1) * 512],
                            start=(kt == 0),
                            stop=(kt == 15),
                            skip_group_check=True,
                        )
                    if kt == 15:
                        # free the at tile fast, then normalize off-psum. In
                        # windows whose fillers clog the DVE FIFO, one ACT
                        # copy (attn rows + sums row, to base 0) releases the
                        # psum tile immediately; h1's rows are then shifted
                        # to partitions 64:128 by an off-critical DVE copy of
                        # SBUF data. Light windows use the direct DVE path.
                        hs = slice(h * HD, (h + 1) * HD)
                        atc = osb.tile([128, QB], f32, tag="atc")
                        srow = osb.tile([1, QB], f32, tag="srow", bufs=1)
                        rbc = osb.tile([128, QB], f32, tag="rbc", bufs=1)
                        if atc_on_act:
                            nc.scalar.activation(
                                atc[0 : HD + 1, :],
                                at[0 : HD + 1, :],
                                mybir.ActivationFunctionType.Copy,
                            )
                            nc.vector.reciprocal(srow[:], atc[HD : HD + 1, :])
                            if h == 1:
                                nc.vector.tensor_copy(
                                    atc[HD : 2 * HD, :], atc[0:HD, :]
                                )
                        else:
                            nc.vector.tensor_copy(atc[hs, :], at[0:HD, :])
                            nc.vector.reciprocal(srow[:], at[HD : HD + 1, :])
                        nc.gpsimd.partition_broadcast(rbc[:], srow[:])
                        nc.vector.tensor_tensor(
                            out=attnT[hs, :],
                            in0=atc[hs, :],
                            in1=rbc[hs, :],
                            op=mult,
                        )

                # at(idx) is emitted one step late so the PE queue runs
                # [at(idx-1), sc(idx+1)] back-to-back the moment exp(idx-1)
                # frees them, with fillers after -- they absorb the wait for
                # exp(idx) instead of delaying sc(idx+1).
                sc_cur = emit_sc(*steps[0])
                for idx, (h, kt) in enumerate(steps):
                    if kt == 0:
                        at_tiles[h] = ps_pool.tile(
                            [HD + 1, QB],
                            f32,
                            tag="at",
                            bufs=1,
                            name=f"at_{b}_{qb}_{h}",
                        )
                    pr = probs_pool.tile([128, QB], f32r, tag="pr")
                    pr_tiles[idx] = pr
                    nc.scalar.activation(pr[:], sc_cur[:], Exp)
                    if idx > 0:
                        emit_at(idx - 1)
                    if idx + 1 < len(steps):
                        sc_cur = emit_sc(*steps[idx + 1])
                    if 1 <= kt <= 14:
                        f = next(fill_iter, None)
                        if f is not None:
                            f()
                emit_at(len(steps) - 1)
                for f in fill_iter:
                    f()
                return attnT

            def outproj_chunks(b, qb, attnT, tail=False):
                """Out-projection for one qi block as 8 filler closures.
                tail=True alternates the psum->sbuf copies between DVE and
                ACT (ACT idles after the last exp) to halve the drain time.
                """
                Copy = mybir.ActivationFunctionType.Copy

                def chunk(st2):
                    def emit():
                        st = qb * (QB // 128) + st2
                        psb = osb.tile([128, E], f32, tag="ptile", bufs=3)
                        for nch in range(2):
                            op = ps_pool.tile([128, 512], f32, tag="u", bufs=2)
                            nc.tensor.matmul(
                                op[:],
                                attnT[:, st2 * 128 : (st2 + 1) * 128],
                                ow_sb[:, nch * 512 : (nch + 1) * 512],
                                start=True,
                                stop=True,
                                skip_group_check=True,
                            )
                            dst = psb[:, nch * 512 : (nch + 1) * 512]
                            if tail and (st2 + nch) % 2 == 0:
                                nc.scalar.activation(dst, op[:], Copy)
                            else:
                                nc.vector.tensor_copy(dst, op[:])
                        # one DMA per 128-row stripe: full 4KB rows halve the
                        # dispatch + DMA-semaphore overhead per byte
                        nc.sync.dma_start(
                            P_dram[b][st * 128 : (st + 1) * 128, :], psb[:]
                        )

                    return emit

                return [chunk(st2) for st2 in range(QB // 128)]

            def reduce_scatter(b):
                if with_cc:
                    nc.gpsimd.collective_compute(
                        "ReduceScatter",
                        add,
                        replica_groups=[list(range(NCORES))],
                        ins=[P_dram[b].opt()],
                        outs=[rs_out[b].opt()],
                    )
                else:  # timeline-sim variant: no collective, copy shard 0
                    nc.sync.dma_start(rs_out[b][:], P_dram[b][0 : S // NCORES, :])
                nc.sync.dma_start(out_ext[b], rs_out[b][:])

            # ---- interleaved schedule: batch-1 projections and batch-0
            # out-projections ride as PE fillers inside the exp-paced
            # attention windows ----
            # Prologue: batch-0 phase-1. Early blocks (0,0),(0,1) route psum
            # copies via the still-idle ACT engine; late blocks (0,2),(0,3)
            # use DVE (ACT is running exps by the time their transposes
            # finish) and emit their K chain first -- window 0 needs kT for
            # every kj block, while their qT columns are only read by
            # window 1. Rope chains alternate Pool/DVE lanes.
            c00 = ph1_chunks(0, 0, act_copies=True)
            c01 = ph1_chunks(0, 1, act_copies=True, rope_dve=True)
            c02 = ph1_chunks(0, 2)
            c03 = ph1_chunks(0, 3, rope_dve=True)
            c00["start"]()
            stage_weights()
            c01["start"]()
            for c in (c00, c01):
                for f in c["tps"]:
                    f()
                c["projk"]()
                c["ropek"]()
                c["projq"]()
                c["ropeq"]()
                c["projv"]()
            c02["start"]()
            c03["start"]()
            for c in (c02, c03):
                for f in c["tps"]:
                    f()
                c["projk"]()
                c["ropek"]()
                c["projv"]()
                c["projq"]()
            for c in (c02, c03):
                c["ropeq"]()

            def block_fillers(c):
                return (
                    [c["start"]]
                    + c["tps"]
                    + [c["projk"], c["ropek"], c["projv"], c["projq"], c["ropeq"]]
                )

            def pair_fillers(ca, cb):
                # both blocks' transposes first (their psum copies queue on
                # DVE), projections well after -- a projection filler that
                # stalls on its copies would block every later score matmul
                # in PE's in-order queue
                return (
                    ca["tps"]
                    + cb["tps"]
                    + [
                        ca["projk"],
                        ca["ropek"],
                        cb["projk"],
                        cb["ropek"],
                        ca["projv"],
                        cb["projv"],
                        ca["projq"],
                        ca["ropeq"],
                        cb["projq"],
                        cb["ropeq"],
                    ]
                )

            c10 = ph1_chunks(1, 0, rope_dve=True)
            c11 = ph1_chunks(1, 1)
            c10["start"]()
            c11["start"]()
            aT00 = attn_chunk(0, 0, fillers=pair_fillers(c10, c11), atc_on_act=True)
            c12 = ph1_chunks(1, 2, rope_dve=True)
            c13 = ph1_chunks(1, 3)
            op00 = outproj_chunks(0, 0, aT00)
            aT01 = attn_chunk(
                0,
                1,
                atc_on_act=True,
                fillers=[c12["start"], c13["start"]]
                + op00[:4]
                + c12["tps"]
                + c13["tps"]
                + [
                    c12["projk"],
                    c12["ropek"],
                    c13["projk"],
                    c13["ropek"],
                    c12["projv"],
                    c13["projv"],
                ],
            )
            # q columns of blocks (1,2),(1,3) are first read by window 3 --
            # their projections/ropes ride window 2 with the out-projections
            aT10 = attn_chunk(
                1,
                0,
                atc_on_act=True,
                fillers=op00[4:]
                + [
                    c12["projq"],
                    c12["ropeq"],
                    c13["projq"],
                    c13["ropeq"],
                ]
                + outproj_chunks(0, 1, aT01),
            )
            reduce_scatter(0)
            aT11 = attn_chunk(1, 1, fillers=outproj_chunks(1, 0, aT10), atc_on_act=True)
            if not with_cc:
                # the shard-0 stand-in copy only reads P rows written by
                # outproj(1,0) above, so it overlaps the tail out-projection
                reduce_scatter(1)
            for f in outproj_chunks(1, 1, aT11, tail=True):
                f()
            if with_cc:
                    nc.gpsimd.collective_compute(
                        "ReduceScatter",
                        add,
                        replica_groups=[list(range(NCORES))],
                        ins=[P_dram[b].opt()],
                        outs=[rs_out[b].opt()],
                    )
                else:  # timeline-sim variant: no collective, copy shard 0
                    nc.sync.dma_start(rs_out[b][:], P_dram[b][0 : S // NCORES, :])
                nc.sync.dma_start(out_ext[b], rs_out[b][:])

            # ---- interleaved schedule: batch-1 projections and batch-0
            # out-projections ride as PE fillers inside the exp-paced
            # attention windows ----
            # Prologue: batch-0 phase-1. Early blocks (0,0),(0,1) route psum
            # copies via the still-idle ACT engine; late blocks (0,2),(0,3)
            # use DVE (ACT is running exps by the time their transposes
            # finish) and emit their K chain first -- window 0 needs kT for
            # every kj block, while their qT columns are only read by
            # window 1. Rope chains alternate Pool/DVE lanes.
            c00 = ph1_chunks(0, 0, act_copies=True)
            c01 = ph1_chunks(0, 1, act_copies=True, rope_dve=True)
            c02 = ph1_chunks(0, 2)
            c03 = ph1_chunks(0, 3, rope_dve=True)
            c00["start"]()
            stage_weights()
            c01["start"]()
            for c in (c00, c01):
                for f in c["tps"]:
                    f()
                c["projk"]()
                c["ropek"]()
                c["projq"]()
                c["ropeq"]()
                c["projv"]()
            c02["start"]()
            c03["start"]()
            for c in (c02, c03):
                for f in c["tps"]:
                    f()
                c["projk"]()
                c["ropek"]()
                c["projv"]()
                c["projq"]()
            for c in (c02, c03):
                c["ropeq"]()

            def block_fillers(c):
                return (
                    [c["start"]]
                    + c["tps"]
                    + [c["projk"], c["ropek"], c["projv"], c["projq"], c["ropeq"]]
                )

            def pair_fillers(ca, cb):
                # both blocks' transposes first (their psum copies queue on
                # DVE), projections well after -- a projection filler that
                # stalls on its copies would block every later score matmul
                # in PE's in-order queue
                return (
                    ca["tps"]
                    + cb["tps"]
                    + [
                        ca["projk"],
                        ca["ropek"],
                        cb["projk"],
                        cb["ropek"],
                        ca["projv"],
                        cb["projv"],
                        ca["projq"],
                        ca["ropeq"],
                        cb["projq"],
                        cb["ropeq"],
                    ]
                )

            c10 = ph1_chunks(1, 0, rope_dve=True)
            c11 = ph1_chunks(1, 1)
            c10["start"]()
            c11["start"]()
            aT00 = attn_chunk(0, 0, fillers=pair_fillers(c10, c11), atc_on_act=True)
            c12 = ph1_chunks(1, 2, rope_dve=True)
            c13 = ph1_chunks(1, 3)
            op00 = outproj_chunks(0, 0, aT00)
            aT01 = attn_chunk(
                0,
                1,
                atc_on_act=True,
                fillers=[c12["start"], c13["start"]]
                + op00[:4]
                + c12["tps"]
                + c13["tps"]
                + [
                    c12["projk"],
                    c12["ropek"],
                    c13["projk"],
                    c13["ropek"],
                    c12["projv"],
                    c13["projv"],
                ],
            )
            # q columns of blocks (1,2),(1,3) are first read by window 3 --
            # their projections/ropes ride window 2 with the out-projections
            aT10 = attn_chunk(
                1,
                0,
                atc_on_act=True,
                fillers=op00[4:]
                + [
                    c12["projq"],
                    c12["ropeq"],
                    c13["projq"],
                    c13["ropeq"],
                ]
                + outproj_chunks(0, 1, aT01),
            )
            reduce_scatter(0)
            aT11 = attn_chunk(1, 1, fillers=outproj_chunks(1, 0, aT10), atc_on_act=True)
            if not with_cc:
                # the shard-0 stand-in copy only reads P rows written by
                # outproj(1,0) above, so it overlaps the tail out-projection
                reduce_scatter(1)
            for f in outproj_chunks(1, 1, aT11, tail=True):
                f()
            if with_cc:
                    nc.gpsimd.collective_compute(
                        "ReduceScatter",
                        add,
                        replica_groups=[list(range(NCORES))],
                        ins=[P_dram[b].opt()],
                        outs=[rs_out[b].opt()],
                    )
                else:  # timeline-sim variant: no collective, copy shard 0
                    nc.sync.dma_start(rs_out[b][:], P_dram[b][0 : S // NCORES, :])
                nc.sync.dma_start(out_ext[b], rs_out[b][:])

            # ---- interleaved schedule: batch-1 projections and batch-0
            # out-projections ride as PE fillers inside the exp-paced
            # attention windows ----
            # Prologue: batch-0 phase-1. Early blocks (0,0),(0,1) route psum
            # copies via the still-idle ACT engine; late blocks (0,2),(0,3)
            # use DVE (ACT is running exps by the time their transposes
            # finish) and emit their K chain first -- window 0 needs kT for
            # every kj block, while their qT columns are only read by
            # window 1. Rope chains alternate Pool/DVE lanes.
            c00 = ph1_chunks(0, 0, act_copies=True)
            c01 = ph1_chunks(0, 1, act_copies=True, rope_dve=True)
            c02 = ph1_chunks(0, 2)
            c03 = ph1_chunks(0, 3, rope_dve=True)
            c00["start"]()
            stage_weights()
            c01["start"]()
            for c in (c00, c01):
                for f in c["tps"]:
                    f()
                c["projk"]()
                c["ropek"]()
                c["projq"]()
                c["ropeq"]()
                c["projv"]()
            c02["start"]()
            c03["start"]()
            for c in (c02, c03):
                for f in c["tps"]:
                    f()
                c["projk"]()
                c["ropek"]()
                c["projv"]()
                c["projq"]()
            for c in (c02, c03):
                c["ropeq"]()

            def block_fillers(c):
                return (
                    [c["start"]]
                    + c["tps"]
                    + [c["projk"], c["ropek"], c["projv"], c["projq"], c["ropeq"]]
                )

            def pair_fillers(ca, cb):
                # both blocks' transposes first (their psum copies queue on
                # DVE), projections well after -- a projection filler that
                # stalls on its copies would block every later score matmul
                # in PE's in-order queue
                return (
                    ca["tps"]
                    + cb["tps"]
                    + [
                        ca["projk"],
                        ca["ropek"],
                        cb["projk"],
                        cb["ropek"],
                        ca["projv"],
                        cb["projv"],
                        ca["projq"],
                        ca["ropeq"],
                        cb["projq"],
                        cb["ropeq"],
                    ]
                )

            c10 = ph1_chunks(1, 0, rope_dve=True)
            c11 = ph1_chunks(1, 1)
            c10["start"]()
            c11["start"]()
            aT00 = attn_chunk(0, 0, fillers=pair_fillers(c10, c11), atc_on_act=True)
            c12 = ph1_chunks(1, 2, rope_dve=True)
            c13 = ph1_chunks(1, 3)
            op00 = outproj_chunks(0, 0, aT00)
            aT01 = attn_chunk(
                0,
                1,
                atc_on_act=True,
                fillers=[c12["start"], c13["start"]]
                + op00[:4]
                + c12["tps"]
                + c13["tps"]
                + [
                    c12["projk"],
                    c12["ropek"],
                    c13["projk"],
                    c13["ropek"],
                    c12["projv"],
                    c13["projv"],
                ],
            )
            # q columns of blocks (1,2),(1,3) are first read by window 3 --
            # their projections/ropes ride window 2 with the out-projections
            aT10 = attn_chunk(
                1,
                0,
                atc_on_act=True,
                fillers=op00[4:]
                + [
                    c12["projq"],
                    c12["ropeq"],
                    c13["projq"],
                    c13["ropeq"],
                ]
                + outproj_chunks(0, 1, aT01),
            )
            reduce_scatter(0)
            aT11 = attn_chunk(1, 1, fillers=outproj_chunks(1, 0, aT10), atc_on_act=True)
            if not with_cc:
                # the shard-0 stand-in copy only reads P rows written by
                # outproj(1,0) above, so it overlaps the tail out-projection
                reduce_scatter(1)
            for f in outproj_chunks(1, 1, aT11, tail=True):
                f()
            if with_cc:
                    nc.gpsimd.collective_compute(
                        "ReduceScatter",
                        add,
                        replica_groups=[list(range(NCORES))],
                        ins=[P_dram[b].opt()],
                        outs=[rs_out[b].opt()],
                    )
                else:  # timeline-sim variant: no collective, copy shard 0
                    nc.sync.dma_start(rs_out[b][:], P_dram[b][0 : S // NCORES, :])
                nc.sync.dma_start(out_ext[b], rs_out[b][:])

            # ---- interleaved schedule: batch-1 projections and batch-0
            # out-projections ride as PE fillers inside the exp-paced
            # attention windows ----
            # Prologue: batch-0 phase-1. Early blocks (0,0),(0,1) route psum
            # copies via the still-idle ACT engine; late blocks (0,2),(0,3)
            # use DVE (ACT is running exps by the time their transposes
            # finish) and emit their K chain first -- window 0 needs kT for
            # every kj block, while their qT columns are only read by
            # window 1. Rope chains alternate Pool/DVE lanes.
            c00 = ph1_chunks(0, 0, act_copies=True)
            c01 = ph1_chunks(0, 1, act_copies=True, rope_dve=True)
            c02 = ph1_chunks(0, 2)
            c03 = ph1_chunks(0, 3, rope_dve=True)
            c00["start"]()
            stage_weights()
            c01["start"]()
            for c in (c00, c01):
                for f in c["tps"]:
                    f()
                c["projk"]()
                c["ropek"]()
                c["projq"]()
                c["ropeq"]()
                c["projv"]()
            c02["start"]()
            c03["start"]()
            for c in (c02, c03):
                for f in c["tps"]:
                    f()
                c["projk"]()
                c["ropek"]()
                c["projv"]()
                c["projq"]()
            for c in (c02, c03):
                c["ropeq"]()

            def block_fillers(c):
                return (
                    [c["start"]]
                    + c["tps"]
                    + [c["projk"], c["ropek"], c["projv"], c["projq"], c["ropeq"]]
                )

            def pair_fillers(ca, cb):
                # both blocks' transposes first (their psum copies queue on
                # DVE), projections well after -- a projection filler that
                # stalls on its copies would block every later score matmul
                # in PE's in-order queue
                return (
                    ca["tps"]
                    + cb["tps"]
                    + [
                        ca["projk"],
                        ca["ropek"],
                        cb["projk"],
                        cb["ropek"],
                        ca["projv"],
                        cb["projv"],
                        ca["projq"],
                        ca["ropeq"],
                        cb["projq"],
                        cb["ropeq"],
                    ]
                )

            c10 = ph1_chunks(1, 0, rope_dve=True)
            c11 = ph1_chunks(1, 1)
            c10["start"]()
            c11["start"]()
            aT00 = attn_chunk(0, 0, fillers=pair_fillers(c10, c11), atc_on_act=True)
            c12 = ph1_chunks(1, 2, rope_dve=True)
            c13 = ph1_chunks(1, 3)
            op00 = outproj_chunks(0, 0, aT00)
            aT01 = attn_chunk(
                0,
                1,
                atc_on_act=True,
                fillers=[c12["start"], c13["start"]]
                + op00[:4]
                + c12["tps"]
                + c13["tps"]
                + [
                    c12["projk"],
                    c12["ropek"],
                    c13["projk"],
                    c13["ropek"],
                    c12["projv"],
                    c13["projv"],
                ],
            )
            # q columns of blocks (1,2),(1,3) are first read by window 3 --
            # their projections/ropes ride window 2 with the out-projections
            aT10 = attn_chunk(
                1,
                0,
                atc_on_act=True,
                fillers=op00[4:]
                + [
                    c12["projq"],
                    c12["ropeq"],
                    c13["projq"],
                    c13["ropeq"],
                ]
                + outproj_chunks(0, 1, aT01),
            )
            reduce_scatter(0)
            aT11 = attn_chunk(1, 1, fillers=outproj_chunks(1, 0, aT10), atc_on_act=True)
            if not with_cc:
                # the shard-0 stand-in copy only reads P rows written by
                # outproj(1,0) above, so it overlaps the tail out-projection
                reduce_scatter(1)
            # tail out-projection: attention is done, so the wide 'sc' psum
            # tiles are free -- both 512-halves of a P stripe land in one
            # tile, one copy (alternating DVE/ACT), one DMA; PE stays warm
            # on a dense mm stream
            Copy = mybir.ActivationFunctionType.Copy
            for st2 in range(QB // 128):
                st = 1 * (QB // 128) + st2
                op = ps_pool.tile([128, QB], f32, tag="sc", bufs=2)
                for nch in range(2):
                    nc.tensor.matmul(
                        op[:, nch * 512 : (nch + 1) * 512],
                        aT11[:, st2 * 128 : (st2 + 1) * 128],
                        ow_sb[:, nch * 512 : (nch + 1) * 512],
                        start=True,
                        stop=True,
                        skip_group_check=True,
                    )
                psb = osb.tile([128, E], f32, tag="ptile", bufs=3)
                if st2 % 2 == 0:
                    nc.scalar.activation(
                        psb[:, 0:512], op[:, 0:512], Copy
                    )
                    nc.vector.tensor_copy(psb[:, 512:1024], op[:, 512:1024])
                else:
                    nc.vector.tensor_copy(psb[:, 0:512], op[:, 0:512])
                    nc.scalar.activation(
                        psb[:, 512:1024], op[:, 512:1024], Copy
                    )
                nc.sync.dma_start(
                    P_dram[1][st * 128 : (st + 1) * 128, :], psb[:]
                )
            if with_cc:
                # the real collective reads all of P(1), so it must be
                # emitted after every P write (program order = dependency)
                reduce_scatter(1)

    nc.compile()
    return nc


def _host_inputs(query, Wq, bq, Wk, bk, Wv, bv, Wo, bo):
    """Per-core input maps (all fp32, C-contiguous)."""
    scaling = HD ** (-0.5)

    invf = 1.0 / (
        10000.0 ** (np.arange(0, HD, 2, dtype=np.float32) / np.float32(HD))
    )
    t = np.arange(S, dtype=np.float32)
    fr = np.outer(t, invf).astype(np.float32)  # [S, 32]
    emb = np.concatenate([fr, fr], axis=1)  # [S, HD]
    cosT = np.cos(emb).T.astype(np.float32)  # [HD, S]
    sinT = np.sin(emb).T.astype(np.float32)
    sign = np.where(np.arange(HD) < HD // 2, -1.0, 1.0).astype(np.float32)[:, None]
    cos_t = np.ascontiguousarray(np.tile(cosT, (HPC, 1)), dtype=np.float32)
    sin_t = np.ascontiguousarray(np.tile(sinT * sign, (HPC, 1)), dtype=np.float32)

    query = np.ascontiguousarray(query, dtype=np.float32)
    in_maps = []
    for c in range(NCORES):
        sl = slice(c * EL, (c + 1) * EL)
        in_maps.append(
            {
                "query": query,
                "q_w": np.ascontiguousarray((Wq[sl, :] * scaling).T, dtype=np.float32),
                "k_w": np.ascontiguousarray(Wk[sl, :].T, dtype=np.float32),
                "v_w": np.ascontiguousarray(Wv[sl, :].T, dtype=np.float32),
                "o_w": np.ascontiguousarray(Wo[:, sl].T, dtype=np.float32),
                "bq_s": np.ascontiguousarray(
                    (bq[sl] * scaling).reshape(EL, 1), dtype=np.float32
                ),
                "bk_s": np.ascontiguousarray(bk[sl].reshape(EL, 1), dtype=np.float32),
                "bv_s": np.ascontiguousarray(bv[sl].reshape(EL, 1), dtype=np.float32),
                "cos_t": cos_t,
                "sin_t": sin_t,
            }
        )
    return in_maps


def kernel(query, Wq, bq, Wk, bk, Wv, bv, Wo, bo):
    global LAST_RESULT
    from concourse.bass_utils import run_bass_kernel_spmd

    if "nc" not in _CACHE:
        _CACHE["nc"] = _build_program()
    nc = _CACHE["nc"]

    in_maps = _host_inputs(
        np.asarray(query),
        np.asarray(Wq),
        np.asarray(bq),
        np.asarray(Wk),
        np.asarray(bk),
        np.asarray(Wv),
        np.asarray(bv),
        np.asarray(Wo),
        np.asarray(bo),
    )
    res = run_bass_kernel_spmd(nc, in_maps, core_ids=list(range(NCORES)))
    LAST_RESULT = res
    shards = [
        res.results[c]["out"].reshape(B, S // NCORES, E).transpose(1, 0, 2)
        for c in range(NCORES)
    ]
    full = np.concatenate(shards, axis=0)
    # output bias is added once on the host (not per-core: the ReduceScatter
    # sums partial projections from all 8 cores)
    return full + np.asarray(bo, dtype=np.float32).reshape(1, 1, E)


# revision 50
# speedup vs baseline: 1.8842x; 1.0060x over previous
"""Trainium2 Bass kernel for nn_LucaGPLMMultiheadAttention.

MHA with RoPE: S=2048, B=2, E=1024, H=16, hd=64, fp32.
Sharding: head-parallel across 8 cores (2 heads x 2 batch = 4 (b,h) pairs
per core). q/k/v projections column-split, out projection row-split with an
on-device ReduceScatter over the sequence axis; host concatenates shards and
adds the output bias (bo) once.

All big matmuls run as float32r (fp32 streamed at full rate when the moving
free dim >= 256; TF32-like rounding, ~3e-4 rel err per matmul).

Schedule (engines execute their instruction streams in order, so emission
order is the schedule):
  - Attention is a software-pipelined exp-paced loop: scores(k+1) is
    emitted before attn(k) so the PE queue never blocks behind exp(k);
    with sc psum double-buffered the cadence is max(ACT ~1.2us, PE).
  - The program interleaves four attention windows (one per (b, qi-block))
    with "filler" closures: batch-1 projections/rope and batch-0
    out-projections execute inside batch-0's exp-paced windows.
  - One PSUM pool fills all 8 banks: u [128,512]x2 (transposes,
    projections, out-proj), sc [128,1024]x2 (scores/exp pipeline),
    at [65,1024]x1 (attn accumulation + ones-column row sums).
  - Engine placement by contention: psum->sbuf copies ride ACT before the
    first exp (prologue blocks 0,1) and DVE afterwards; rope chains run
    entirely on one engine (Pool or DVE, alternating per block) since
    GPSIMD tensor ops are accepted as f32r rounding producers; the
    attn-psum tile is released via a single ACT copy in filler-heavy
    windows so it never queues behind the DVE FIFO.
  - Softmax denominators come from a ones column appended to the v tiles;
    normalization multiplies the reciprocal into attnT (the out-proj
    stationary operand), fused with the psum read.
bo is added on the host after the shard gather (the ReduceScatter sums
partial projections, so a per-core bias would be counted 8 times).
"""

import os
import sys

sys.path.insert(0, "/opt/trn_rl_repo")

import numpy as np

S = 2048
B = 2
E = 1024
H = 16
HD = 64
NCORES = 8
HPC = H // NCORES  # heads per core = 2
EL = HPC * HD  # local embed slice = 128
SB = S * B  # 4096 rows
SHARD = SB // NCORES  # 512 rows per core after reduce-scatter
QB = 1024  # qi block size

_CACHE: dict = {}
LAST_RESULT = None


def _build_program(with_cc: bool = True):
    import concourse.mybir as mybir
    import concourse.tile as tile
    from concourse import bacc
    from concourse.masks import make_identity

    f32 = mybir.dt.float32
    f32r = mybir.dt.float32r
    Exp = mybir.ActivationFunctionType.Exp
    add = mybir.AluOpType.add
    mult = mybir.AluOpType.mult

    nc = bacc.Bacc(
        "TRN2",
        target_bir_lowering=False,
        debug=False,
        enable_asserts=False,
        num_devices=NCORES,
    )

    def din(name, shape):
        return nc.dram_tensor(name, shape, f32, kind="ExternalInput").ap()

    query = din("query", [S, B, E])
    q_w = din("q_w", [E, EL])  # (Wq_slice * scaling).T
    k_w = din("k_w", [E, EL])
    v_w = din("v_w", [E, EL])
    o_w = din("o_w", [EL, E])  # Wo[:, slice].T
    bq_s = din("bq_s", [EL, 1])
    bk_s = din("bk_s", [EL, 1])
    bv_s = din("bv_s", [EL, 1])
    cos_t = din("cos_t", [EL, S])  # 2-head stacked rope tables (sin sign-folded)
    sin_t = din("sin_t", [EL, S])
    # b-major shard layout: the reduce-scatter (or its no-collective stand-in)
    # writes its contiguous [S/NCORES, E] result straight into the external
    # output -- no final reshuffle DMA. The host reassembles shards.
    out_ext = nc.dram_tensor(
        "out", [B, S // NCORES, E], f32, kind="ExternalOutput"
    ).ap()

    with tile.TileContext(nc) as tc:
        with (
            tc.tile_pool(name="const", bufs=1) as const,
            tc.tile_pool(name="persist", bufs=1) as persist,
            tc.tile_pool(name="dram", bufs=1, space="DRAM") as dram,
            # one PSUM pool, 16KB/partition exactly:
            #   u  [128,512] f32 x2 = 4KB  (transposes, projections, out-proj)
            #   sc [128,1024] f32 x2 = 8KB (scores / exp pipeline)
            #   at [65,1024] f32 x1 = 4KB  (attn accumulation + row sums)
            tc.tile_pool(name="ps", bufs=1, space="PSUM") as ps_pool,
            tc.tile_pool(name="ld", bufs=4) as ld,
            tc.tile_pool(name="qtb", bufs=2) as qtb,
            tc.tile_pool(name="vblk", bufs=1) as vblk_pool,
            tc.tile_pool(name="rope", bufs=1) as rope,
            tc.tile_pool(name="probs", bufs=3) as probs_pool,
            tc.tile_pool(name="attn_sb", bufs=2) as attn_sb,
            tc.tile_pool(name="osb", bufs=2) as osb,
        ):
            # ---- constants to SBUF (weights staged fp32, rounded to f32r) ----
            qw_sb = const.tile([128, 8, EL], f32r, name="qw_sb")
            kw_sb = const.tile([128, 8, EL], f32r, name="kw_sb")
            vw_sb = const.tile([128, 8, EL], f32r, name="vw_sb")
            ow_sb = const.tile([EL, E], f32r, name="ow_sb")
            bq_sb = const.tile([EL, 1], f32, name="bq_sb")
            bk_sb = const.tile([EL, 1], f32, name="bk_sb")
            bv_sb = const.tile([EL, 1], f32, name="bv_sb")
            ident = const.tile([128, 128], f32, name="ident")
            # two I_64 stacked on partitions 0:64 and 64:128 (for h=1 transposes)
            id64r = const.tile([128, HD], f32r, name="id64r")
            cos_sb = const.tile([EL, S], f32, name="cos_sb")
            sin_sb = const.tile([EL, S], f32, name="sin_sb")

            def stage_weights():
                # emitted after the first query-block DMAs so the query (on
                # the critical path to the first transposes) transfers first;
                # o_w last (first needed only by the out-projection ~60us in)
                with tc.tile_pool(name="wstage", bufs=1) as wstage:
                    for src, dst in ((q_w, qw_sb), (k_w, kw_sb)):
                        stg = wstage.tile([128, 8, EL], f32, tag="wstg")
                        nc.sync.dma_start(
                            stg[:], src.rearrange("(c p) m -> p c m", p=128)
                        )
                        nc.vector.tensor_copy(dst[:], stg[:])
                    nc.sync.dma_start(bq_sb[:], bq_s[:])
                    nc.sync.dma_start(bk_sb[:], bk_s[:])
                    nc.sync.dma_start(cos_sb[:], cos_t[:])
                    nc.sync.dma_start(sin_sb[:], sin_t[:])
                    stg = wstage.tile([128, 8, EL], f32, tag="wstg")
                    nc.sync.dma_start(
                        stg[:], v_w.rearrange("(c p) m -> p c m", p=128)
                    )
                    nc.vector.tensor_copy(vw_sb[:], stg[:])
                    nc.sync.dma_start(bv_sb[:], bv_s[:])
                    stg = wstage.tile([EL, E], f32, tag="owstg")
                    nc.sync.dma_start(stg[:], o_w[:])
                    nc.vector.tensor_copy(ow_sb[:], stg[:])

            make_identity(nc, ident[:])
            nc.vector.tensor_copy(id64r[0:HD, :], ident[0:HD, 0:HD])
            nc.vector.tensor_copy(id64r[HD:128, :], ident[0:HD, 0:HD])

            # ---- persistent activations ----
            qT = persist.tile([EL, SB], f32r, name="qT")  # [2h*hd, b-major cols]
            kT = persist.tile([EL, SB], f32r, name="kT")
            # v kj-tiles [128, 64] + ones column, built in ph1
            vaug = persist.tile([128, HPC * B * 16, HD + 1], f32r, name="vaug")
            ones_f = const.tile([128, HPC * B * 16], f32, name="ones_f")
            nc.vector.memset(ones_f[:], 1.0)
            nc.vector.tensor_copy(vaug[:, :, HD], ones_f[:])

            P_dram = [dram.tile([S, E], f32, name=f"P_dram{b}") for b in range(B)]
            rs_out = [
                dram.tile([S // NCORES, E], f32, name=f"rs_out{b}")
                for b in range(B)
            ]


            # ---- phase-1: transpose query, project, rope, v-tiles ----
            # Emitted as closures per 512-row block so the work can be
            # injected into attention chunks as PE fillers. Rope closures are
            # separated from projections and emitted last: the rope-add (DVE)
            # waits on Pool multiplies, and sitting in the middle of the DVE
            # FIFO it head-of-line blocks every later block's psum copies.
            # act_copies=True routes the psum->sbuf copies to the (otherwise
            # idle) ACT engine -- used in the prologue, before the first exp.
            def ph1_chunks(b, sblk, act_copies=False, rope_dve=False):
                col0 = b * S + sblk * 512
                state = {}

                def copy_engine(dst, src):
                    if act_copies:
                        nc.scalar.activation(
                            dst, src, mybir.ActivationFunctionType.Copy
                        )
                    else:
                        nc.vector.tensor_copy(dst, src)

                def c_start():
                    state["qt"] = qtb.tile(
                        [128, 8, 512], f32r, tag="qt_blk", name=f"qt_{b}_{sblk}"
                    )
                    state["v"] = vblk_pool.tile(
                        [128, 512], f32r, tag="v_blk", name=f"v_{b}_{sblk}"
                    )
                    state["halves"] = []
                    for hf in range(2):
                        qry = ld.tile([128, 2, E], f32, tag="qry")
                        nc.sync.dma_start(
                            qry[:],
                            query[
                                sblk * 512 + hf * 256 : sblk * 512
                                + (hf + 1) * 256,
                                b,
                            ].rearrange("(i p) e -> p i e", p=128),
                        )
                        state["halves"].append(qry)

                def c_tp(i, eg):
                    # fp32 transposes: the verifier requires f32r matmult
                    # operands to come from a rounding instruction, so the
                    # raw DMA'd query stays fp32
                    qry = state["halves"][i // 2]
                    ii = i % 2
                    qt_blk = state["qt"]
                    tp = ps_pool.tile([128, 512], f32, tag="u", bufs=2)
                    for ec2 in range(4):
                        ec = eg * 4 + ec2
                        nc.tensor.transpose(
                            tp[:, ec2 * 128 : (ec2 + 1) * 128],
                            qry[:, ii, ec * 128 : (ec + 1) * 128],
                            ident[:],
                        )
                    copy_engine(
                        qt_blk[
                            :, eg * 4 : (eg + 1) * 4, i * 128 : (i + 1) * 128
                        ],
                        tp[:].rearrange("p (c m) -> p c m", c=4),
                    )

                def c_proj(which):
                    w_sb, bias = (
                        (qw_sb, bq_sb),
                        (kw_sb, bk_sb),
                        (vw_sb, bv_sb),
                    )[which]
                    dst = (
                        qT[:, col0 : col0 + 512],
                        kT[:, col0 : col0 + 512],
                        state["v"][:],
                    )[which]
                    qt_blk = state["qt"]
                    pj = ps_pool.tile([128, 512], f32, tag="u", bufs=2)
                    for ec in range(8):
                        nc.tensor.matmul(
                            pj[:],
                            w_sb[:, ec, :],
                            qt_blk[:, ec, :],
                            start=(ec == 0),
                            stop=(ec == 7),
                        )
                    nc.vector.tensor_scalar_add(dst, pj[:], bias[:])
                    if which == 2:
                        # v natural kj-tiles for this block (f32r transpose:
                        # v_blk is DVE-rounded, so 1.5 cycles/row)
                        v_blk = state["v"]
                        for kt2 in range(4):
                            kt = sblk * 4 + kt2
                            for h in range(HPC):
                                vt = ps_pool.tile([128, 512], f32, tag="u", bufs=2)
                                vtr = vt[:, 0:HD].bitcast(f32r)
                                nc.tensor.transpose(
                                    vtr,
                                    v_blk[
                                        h * HD : (h + 1) * HD,
                                        kt2 * 128 : (kt2 + 1) * 128,
                                    ],
                                    id64r[h * HD : (h + 1) * HD, :],
                                )
                                copy_engine(
                                    vaug[:, (h * B + b) * 16 + kt, :HD], vtr
                                )

                def c_rope(which):
                    # rope: x' = x*cos + shuffle(x)*sin_f (sign folded in
                    # sin). Shuffle via partition-shifted copies (walrus
                    # requires TensorTensor operands to share a start
                    # partition; only copies may shift). rope_dve picks the
                    # engine lane so concurrent blocks' ropes don't queue
                    # behind each other.
                    eng = nc.vector if rope_dve else nc.gpsimd
                    dst = (qT, kT)[which][:, col0 : col0 + 512]
                    ccol = slice(sblk * 512, (sblk + 1) * 512)
                    t1 = rope.tile([EL, 512], f32, tag="t1")
                    t2 = rope.tile([EL, 512], f32, tag="t2")
                    for h in range(HPC):
                        p0 = h * HD
                        eng.tensor_copy(
                            t2[p0 : p0 + 32, :], dst[p0 + 32 : p0 + 64, :]
                        )
                        eng.tensor_copy(
                            t2[p0 + 32 : p0 + 64, :], dst[p0 : p0 + 32, :]
                        )
                    eng.tensor_tensor(
                        out=t1[:], in0=dst, in1=cos_sb[:, ccol], op=mult
                    )
                    eng.tensor_tensor(
                        out=t2[:], in0=t2[:], in1=sin_sb[:, ccol], op=mult
                    )
                    eng.tensor_tensor(out=dst, in0=t1[:], in1=t2[:], op=add)

                return {
                    "start": c_start,
                    "tps": [
                        lambda i=i, eg=eg: c_tp(i, eg)
                        for i in range(4)
                        for eg in range(2)
                    ],
                    "projq": lambda: c_proj(0),
                    "projk": lambda: c_proj(1),
                    "projv": lambda: c_proj(2),
                    "ropeq": lambda: c_rope(0),
                    "ropek": lambda: c_rope(1),
                }

            # ---- attention chunk: one qi block (both heads), normalized ----
            # Software-pipelined: scores(step+1) is emitted BEFORE the attn
            # matmuls of the current step, so the in-order PE queue never
            # blocks behind exp(step) -- the cadence becomes max(ACT, PE)
            # instead of their serial sum. The finished at-psum tile is
            # released by one immediate DVE copy; normalize runs off-psum.
            # `fillers` are PE-side work chunks (out-projection) injected
            # mid-loop to fill exp-paced PE idle without touching ACT.
            def attn_chunk(b, qb, fillers=(), atc_on_act=False):
                q0 = b * S + qb * QB
                attnT = attn_sb.tile([EL, QB], f32r, tag="attnT")
                steps = [(h, kt) for h in range(HPC) for kt in range(16)]

                def emit_sc(h, kt):
                    hs = slice(h * HD, (h + 1) * HD)
                    k0 = b * S + kt * 128
                    sc = ps_pool.tile([128, QB], f32, tag="sc", bufs=2)
                    for half in range(2):
                        nc.tensor.matmul(
                            sc[:, half * 512 : (half + 1) * 512],
                            kT[hs, k0 : k0 + 128],
                            qT[hs, q0 + half * 512 : q0 + (half + 1) * 512],
                            start=True,
                            stop=True,
                            skip_group_check=True,
                        )
                    return sc

                fill_iter = iter(fillers)
                at_tiles = [None, None]
                pr_tiles = {}

                def emit_at(idx):
                    h, kt = steps[idx]
                    pair = h * B + b
                    at = at_tiles[h]
                    pr = pr_tiles.pop(idx)
                    for half in range(2):
                        nc.tensor.matmul(
                            at[:, half * 512 : (half + 1) * 512],
                            vaug[:, pair * 16 + kt, :],
                            pr[:, half * 512 : (half + 1) * 512],
                            start=(kt == 0),
                            stop=(kt == 15),
                            skip_group_check=True,
                        )
                    if kt == 15:
                        # free the at tile fast, then normalize off-psum. In
                        # windows whose fillers clog the DVE FIFO, one ACT
                        # copy (attn rows + sums row, to base 0) releases the
                        # psum tile immediately; h1's rows are then shifted
                        # to partitions 64:128 by an off-critical DVE copy of
                        # SBUF data. Light windows use the direct DVE path.
                        hs = slice(h * HD, (h + 1) * HD)
                        atc = osb.tile([128, QB], f32, tag="atc")
                        srow = osb.tile([1, QB], f32, tag="srow", bufs=1)
                        rbc = osb.tile([128, QB], f32, tag="rbc", bufs=1)
                        if atc_on_act:
                            nc.scalar.activation(
                                atc[0 : HD + 1, :],
                                at[0 : HD + 1, :],
                                mybir.ActivationFunctionType.Copy,
                            )
                            nc.vector.reciprocal(srow[:], atc[HD : HD + 1, :])
                            if h == 1:
                                nc.vector.tensor_copy(
                                    atc[HD : 2 * HD, :], atc[0:HD, :]
                                )
                        else:
                            nc.vector.tensor_copy(atc[hs, :], at[0:HD, :])
                            nc.vector.reciprocal(srow[:], at[HD : HD + 1, :])
                        nc.gpsimd.partition_broadcast(rbc[:], srow[:])
                        nc.vector.tensor_tensor(
                            out=attnT[hs, :],
                            in0=atc[hs, :],
                            in1=rbc[hs, :],
                            op=mult,
                        )

                # at(idx) is emitted one step late so the PE queue runs
                # [at(idx-1), sc(idx+1)] back-to-back the moment exp(idx-1)
                # frees them, with fillers after -- they absorb the wait for
                # exp(idx) instead of delaying sc(idx+1).
                sc_cur = emit_sc(*steps[0])
                for idx, (h, kt) in enumerate(steps):
                    if kt == 0:
                        at_tiles[h] = ps_pool.tile(
                            [HD + 1, QB],
                            f32,
                            tag="at",
                            bufs=1,
                            name=f"at_{b}_{qb}_{h}",
                        )
                    pr = probs_pool.tile([128, QB], f32r, tag="pr")
                    pr_tiles[idx] = pr
                    nc.scalar.activation(pr[:], sc_cur[:], Exp)
                    if idx > 0:
                        emit_at(idx - 1)
                    if idx + 1 < len(steps):
                        sc_cur = emit_sc(*steps[idx + 1])
                    if 1 <= kt <= 14:
                        f = next(fill_iter, None)
                        if f is not None:
                            f()
                emit_at(len(steps) - 1)
                for f in fill_iter:
                    f()
                return attnT

            def outproj_chunks(b, qb, attnT, tail=False):
                """Out-projection for one qi block as 8 filler closures.
                tail=True alternates the psum->sbuf copies between DVE and
                ACT (ACT idles after the last exp) to halve the drain time.
                """
                Copy = mybir.ActivationFunctionType.Copy

                def chunk(st2):
                    def emit():
                        st = qb * (QB // 128) + st2
                        psb = osb.tile([128, E], f32, tag="ptile", bufs=3)
                        for nch in range(2):
                            op = ps_pool.tile([128, 512], f32, tag="u", bufs=2)
                            nc.tensor.matmul(
                                op[:],
                                attnT[:, st2 * 128 : (st2 + 1) * 128],
                                ow_sb[:, nch * 512 : (nch + 1) * 512],
                                start=True,
                                stop=True,
                                skip_group_check=True,
                            )
                            dst = psb[:, nch * 512 : (nch + 1) * 512]
                            if tail and (st2 + nch) % 2 == 0:
                                nc.scalar.activation(dst, op[:], Copy)
                            else:
                                nc.vector.tensor_copy(dst, op[:])
                        # one DMA per 128-row stripe: full 4KB rows halve the
                        # dispatch + DMA-semaphore overhead per byte
                        nc.sync.dma_start(
                            P_dram[b][st * 128 : (st + 1) * 128, :], psb[:]
                        )

                    return emit

                return [chunk(st2) for st2 in range(QB // 128)]

            def reduce_scatter(b):
                if with_cc:
                    nc.gpsimd.collective_compute(
                        "ReduceScatter",
                        add,
                        replica_groups=[list(range(NCORES))],
                        ins=[P_dram[b].opt()],
                        outs=[rs_out[b].opt()],
                    )
                else:  # timeline-sim variant: no collective, copy shard 0
                    nc.sync.dma_start(rs_out[b][:], P_dram[b][0 : S // NCORES, :])
                nc.sync.dma_start(out_ext[b], rs_out[b][:])

            # ---- interleaved schedule: batch-1 projections and batch-0
            # out-projections ride as PE fillers inside the exp-paced
            # attention windows ----
            # Prologue: batch-0 phase-1. Early blocks (0,0),(0,1) route psum
            # copies via the still-idle ACT engine; late blocks (0,2),(0,3)
            # use DVE (ACT is running exps by the time their transposes
            # finish) and emit their K chain first -- window 0 needs kT for
            # every kj block, while their qT columns are only read by
            # window 1. Rope chains alternate Pool/DVE lanes.
            c00 = ph1_chunks(0, 0, act_copies=True)
            c01 = ph1_chunks(0, 1, act_copies=True, rope_dve=True)
            c02 = ph1_chunks(0, 2)
            c03 = ph1_chunks(0, 3, rope_dve=True)
            c00["start"]()
            stage_weights()
            c01["start"]()
            for c in (c00, c01):
                for f in c["tps"]:
                    f()
                c["projk"]()
                c["ropek"]()
                c["projq"]()
                c["ropeq"]()
                c["projv"]()
            c02["start"]()
            c03["start"]()
            for c in (c02, c03):
                for f in c["tps"]:
                    f()
                c["projk"]()
                c["ropek"]()
                c["projv"]()
                c["projq"]()
            for c in (c02, c03):
                c["ropeq"]()

            def block_fillers(c):
                return (
                    [c["start"]]
                    + c["tps"]
                    + [c["projk"], c["ropek"], c["projv"], c["projq"], c["ropeq"]]
                )

            def pair_fillers(ca, cb):
                # both blocks' transposes first (their psum copies queue on
                # DVE), projections well after -- a projection filler that
                # stalls on its copies would block every later score matmul
                # in PE's in-order queue
                return (
                    ca["tps"]
                    + cb["tps"]
                    + [
                        ca["projk"],
                        ca["ropek"],
                        cb["projk"],
                        cb["ropek"],
                        ca["projv"],
                        cb["projv"],
                        ca["projq"],
                        ca["ropeq"],
                        cb["projq"],
                        cb["ropeq"],
                    ]
                )

            c10 = ph1_chunks(1, 0, rope_dve=True)
            c11 = ph1_chunks(1, 1)
            c10["start"]()
            c11["start"]()
            aT00 = attn_chunk(0, 0, fillers=pair_fillers(c10, c11), atc_on_act=True)
            c12 = ph1_chunks(1, 2, rope_dve=True)
            c13 = ph1_chunks(1, 3)
            op00 = outproj_chunks(0, 0, aT00)
            aT01 = attn_chunk(
                0,
                1,
                atc_on_act=True,
                fillers=[c12["start"], c13["start"]]
                + op00[:4]
                + c12["tps"]
                + c13["tps"]
                + [
                    c12["projk"],
                    c12["ropek"],
                    c13["projk"],
                    c13["ropek"],
                    c12["projv"],
                    c13["projv"],
                ],
            )
            # q columns of blocks (1,2),(1,3) are first read by window 3 --
            # their projections/ropes ride window 2 with the out-projections
            aT10 = attn_chunk(
                1,
                0,
                atc_on_act=True,
                fillers=op00[4:]
                + [
                    c12["projq"],
                    c12["ropeq"],
                    c13["projq"],
                    c13["ropeq"],
                ]
                + outproj_chunks(0, 1, aT01),
            )
            reduce_scatter(0)
            aT11 = attn_chunk(1, 1, fillers=outproj_chunks(1, 0, aT10), atc_on_act=True)
            if not with_cc:
                # the shard-0 stand-in copy only reads P rows written by
                # outproj(1,0) above, so it overlaps the tail out-projection
                reduce_scatter(1)
            for f in outproj_chunks(1, 1, aT11, tail=True):
                f()
            if with_cc:
                    nc.gpsimd.collective_compute(
                        "ReduceScatter",
                        add,
                        replica_groups=[list(range(NCORES))],
                        ins=[P_dram[b].opt()],
                        outs=[rs_out[b].opt()],
                    )
                else:  # timeline-sim variant: no collective, copy shard 0
                    nc.sync.dma_start(rs_out[b][:], P_dram[b][0 : S // NCORES, :])
                nc.sync.dma_start(out_ext[b], rs_out[b][:])

            # ---- interleaved schedule: batch-1 projections and batch-0
            # out-projections ride as PE fillers inside the exp-paced
            # attention windows ----
            # Prologue: batch-0 phase-1. Early blocks (0,0),(0,1) route psum
            # copies via the still-idle ACT engine; late blocks (0,2),(0,3)
            # use DVE (ACT is running exps by the time their transposes
            # finish) and emit their K chain first -- window 0 needs kT for
            # every kj block, while their qT columns are only read by
            # window 1. Rope chains alternate Pool/DVE lanes.
            c00 = ph1_chunks(0, 0, act_copies=True)
            c01 = ph1_chunks(0, 1, act_copies=True, rope_dve=True)
            c02 = ph1_chunks(0, 2)
            c03 = ph1_chunks(0, 3, rope_dve=True)
            c00["start"]()
            stage_weights()
            c01["start"]()
            for c in (c00, c01):
                for f in c["tps"]:
                    f()
                c["projk"]()
                c["ropek"]()
                c["projq"]()
                c["ropeq"]()
                c["projv"]()
            c02["start"]()
            c03["start"]()
            for c in (c02, c03):
                for f in c["tps"]:
                    f()
                c["projk"]()
                c["ropek"]()
                c["projv"]()
                c["projq"]()
            for c in (c02, c03):
                c["ropeq"]()

            def block_fillers(c):
                return (
                    [c["start"]]
                    + c["tps"]
                    + [c["projk"], c["ropek"], c["projv"], c["projq"], c["ropeq"]]
                )

            def pair_fillers(ca, cb):
                # both blocks' transposes first (their psum copies queue on
                # DVE), projections well after -- a projection filler that
                # stalls on its copies would block every later score matmul
                # in PE's in-order queue
                return (
                    ca["tps"]
                    + cb["tps"]
                    + [
                        ca["projk"],
                        ca["ropek"],
                        cb["projk"],
                        cb["ropek"],
                        ca["projv"],
                        cb["projv"],
                        ca["projq"],
                        ca["ropeq"],
                        cb["projq"],
                        cb["ropeq"],
                    ]
                )

            c10 = ph1_chunks(1, 0, rope_dve=True)
            c11 = ph1_chunks(1, 1)
            c10["start"]()
            c11["start"]()
            aT00 = attn_chunk(0, 0, fillers=pair_fillers(c10, c11), atc_on_act=True)
            c12 = ph1_chunks(1, 2, rope_dve=True)
            c13 = ph1_chunks(1, 3)
            op00 = outproj_chunks(0, 0, aT00)
            aT01 = attn_chunk(
                0,
                1,
                atc_on_act=True,
                fillers=[c12["start"], c13["start"]]
                + op00[:4]
                + c12["tps"]
                + c13["tps"]
                + [
                    c12["projk"],
                    c12["ropek"],
                    c13["projk"],
                    c13["ropek"],
                    c12["projv"],
                    c13["projv"],
                ],
            )
            # q columns of blocks (1,2),(1,3) are first read by window 3 --
            # their projections/ropes ride window 2 with the out-projections
            aT10 = attn_chunk(
                1,
                0,
                atc_on_act=True,
                fillers=op00[4:]
                + [
                    c12["projq"],
                    c12["ropeq"],
                    c13["projq"],
                    c13["ropeq"],
                ]
                + outproj_chunks(0, 1, aT01),
            )
            reduce_scatter(0)
            aT11 = attn_chunk(1, 1, fillers=outproj_chunks(1, 0, aT10), atc_on_act=True)
            if not with_cc:
                # the shard-0 stand-in copy only reads P rows written by
                # outproj(1,0) above, so it overlaps the tail out-projection
                reduce_scatter(1)
            for f in outproj_chunks(1, 1, aT11, tail=True):
                f()
            if with_cc:
                    nc.gpsimd.collective_compute(
                        "ReduceScatter",
                        add,
                        replica_groups=[list(range(NCORES))],
                        ins=[P_dram[b].opt()],
                        outs=[rs_out[b].opt()],
                    )
                else:  # timeline-sim variant: no collective, copy shard 0
                    nc.sync.dma_start(rs_out[b][:], P_dram[b][0 : S // NCORES, :])
                nc.sync.dma_start(out_ext[b], rs_out[b][:])

            # ---- interleaved schedule: batch-1 projections and batch-0
            # out-projections ride as PE fillers inside the exp-paced
            # attention windows ----
            # Prologue: batch-0 phase-1. Early blocks (0,0),(0,1) route psum
            # copies via the still-idle ACT engine; late blocks (0,2),(0,3)
            # use DVE (ACT is running exps by the time their transposes
            # finish) and emit their K chain first -- window 0 needs kT for
            # every kj block, while their qT columns are only read by
            # window 1. Rope chains alternate Pool/DVE lanes.
            c00 = ph1_chunks(0, 0, act_copies=True)
            c01 = ph1_chunks(0, 1, act_copies=True, rope_dve=True)
            c02 = ph1_chunks(0, 2)
            c03 = ph1_chunks(0, 3, rope_dve=True)
            c00["start"]()
            stage_weights()
            c01["start"]()
            for c in (c00, c01):
                for f in c["tps"]:
                    f()
                c["projk"]()
                c["ropek"]()
                c["projq"]()
                c["ropeq"]()
                c["projv"]()
            c02["start"]()
            c03["start"]()
            for c in (c02, c03):
                for f in c["tps"]:
                    f()
                c["projk"]()
                c["ropek"]()
                c["projv"]()
                c["projq"]()
            for c in (c02, c03):
                c["ropeq"]()

            def block_fillers(c):
                return (
                    [c["start"]]
                    + c["tps"]
                    + [c["projk"], c["ropek"], c["projv"], c["projq"], c["ropeq"]]
                )

            def pair_fillers(ca, cb):
                # both blocks' transposes first (their psum copies queue on
                # DVE), projections well after -- a projection filler that
                # stalls on its copies would block every later score matmul
                # in PE's in-order queue
                return (
                    ca["tps"]
                    + cb["tps"]
                    + [
                        ca["projk"],
                        ca["ropek"],
                        cb["projk"],
                        cb["ropek"],
                        ca["projv"],
                        cb["projv"],
                        ca["projq"],
                        ca["ropeq"],
                        cb["projq"],
                        cb["ropeq"],
                    ]
                )

            c10 = ph1_chunks(1, 0, rope_dve=True)
            c11 = ph1_chunks(1, 1)
            c10["start"]()
            c11["start"]()
            aT00 = attn_chunk(0, 0, fillers=pair_fillers(c10, c11), atc_on_act=True)
            c12 = ph1_chunks(1, 2, rope_dve=True)
            c13 = ph1_chunks(1, 3)
            op00 = outproj_chunks(0, 0, aT00)
            aT01 = attn_chunk(
                0,
                1,
                atc_on_act=True,
                fillers=[c12["start"], c13["start"]]
                + op00[:4]
                + c12["tps"]
                + c13["tps"]
                + [
                    c12["projk"],
                    c12["ropek"],
                    c13["projk"],
                    c13["ropek"],
                    c12["projv"],
                    c13["projv"],
                ],
            )
            # q columns of blocks (1,2),(1,3) are first read by window 3 --
            # their projections/ropes ride window 2 with the out-projections
            aT10 = attn_chunk(
                1,
                0,
                atc_on_act=True,
                fillers=op00[4:]
                + [
                    c12["projq"],
                    c12["ropeq"],
                    c13["projq"],
                    c13["ropeq"],
                ]
                + outproj_chunks(0, 1, aT01),
            )
            reduce_scatter(0)
            aT11 = attn_chunk(1, 1, fillers=outproj_chunks(1, 0, aT10), atc_on_act=True)
            if not with_cc:
                # the shard-0 stand-in copy only reads P rows written by
                # outproj(1,0) above, so it overlaps the tail out-projection
                reduce_scatter(1)
            for f in outproj_chunks(1, 1, aT11, tail=True):
                f()
            if with_cc:
                    nc.gpsimd.collective_compute(
                        "ReduceScatter",
                        add,
                        replica_groups=[list(range(NCORES))],
                        ins=[P_dram[b].opt()],
                        outs=[rs_out[b].opt()],
                    )
                else:  # timeline-sim variant: no collective, copy shard 0
                    nc.sync.dma_start(rs_out[b][:], P_dram[b][0 : S // NCORES, :])
                nc.sync.dma_start(out_ext[b], rs_out[b][:])

            # ---- interleaved schedule: batch-1 projections and batch-0
            # out-projections ride as PE fillers inside the exp-paced
            # attention windows ----
            # Prologue: batch-0 phase-1. Early blocks (0,0),(0,1) route psum
            # copies via the still-idle ACT engine; late blocks (0,2),(0,3)
            # use DVE (ACT is running exps by the time their transposes
            # finish) and emit their K chain first -- window 0 needs kT for
            # every kj block, while their qT columns are only read by
            # window 1. Rope chains alternate Pool/DVE lanes.
            c00 = ph1_chunks(0, 0, act_copies=True)
            c01 = ph1_chunks(0, 1, act_copies=True, rope_dve=True)
            c02 = ph1_chunks(0, 2)
            c03 = ph1_chunks(0, 3, rope_dve=True)
            c00["start"]()
            stage_weights()
            c01["start"]()
            for c in (c00, c01):
                for f in c["tps"]:
                    f()
                c["projk"]()
                c["ropek"]()
                c["projq"]()
                c["ropeq"]()
                c["projv"]()
            c02["start"]()
            c03["start"]()
            for c in (c02, c03):
                for f in c["tps"]:
                    f()
                c["projk"]()
                c["ropek"]()
                c["projv"]()
                c["projq"]()
            for c in (c02, c03):
                c["ropeq"]()

            def block_fillers(c):
                return (
                    [c["start"]]
                    + c["tps"]
                    + [c["projk"], c["ropek"], c["projv"], c["projq"], c["ropeq"]]
                )

            def pair_fillers(ca, cb):
                # both blocks' transposes first (their psum copies queue on
                # DVE), projections well after -- a projection filler that
                # stalls on its copies would block every later score matmul
                # in PE's in-order queue
                return (
                    ca["tps"]
                    + cb["tps"]
                    + [
                        ca["projk"],
                        ca["ropek"],
                        cb["projk"],
                        cb["ropek"],
                        ca["projv"],
                        cb["projv"],
                        ca["projq"],
                        ca["ropeq"],
                        cb["projq"],
                        cb["ropeq"],
                    ]
                )

            c10 = ph1_chunks(1, 0, rope_dve=True)
            c11 = ph1_chunks(1, 1)
            c10["start"]()
            c11["start"]()
            aT00 = attn_chunk(0, 0, fillers=pair_fillers(c10, c11), atc_on_act=True)
            c12 = ph1_chunks(1, 2, rope_dve=True)
            c13 = ph1_chunks(1, 3)
            op00 = outproj_chunks(0, 0, aT00)
            aT01 = attn_chunk(
                0,
                1,
                atc_on_act=True,
                fillers=[c12["start"], c13["start"]]
                + op00[:4]
                + c12["tps"]
                + c13["tps"]
                + [
                    c12["projk"],
                    c12["ropek"],
                    c13["projk"],
                    c13["ropek"],
                    c12["projv"],
                    c13["projv"],
                ],
            )
            # q columns of blocks (1,2),(1,3) are first read by window 3 --
            # their projections/ropes ride window 2 with the out-projections
            aT10 = attn_chunk(
                1,
                0,
                atc_on_act=True,
                fillers=op00[4:]
                + [
                    c12["projq"],
                    c12["ropeq"],
                    c13["projq"],
                    c13["ropeq"],
                ]
                + outproj_chunks(0, 1, aT01),
            )
            reduce_scatter(0)
            aT11 = attn_chunk(1, 1, fillers=outproj_chunks(1, 0, aT10), atc_on_act=True)
            if not with_cc:
                # the shard-0 stand-in copy only reads P rows written by
                # outproj(1,0) above, so it overlaps the tail out-projection
                reduce_scatter(1)
            # tail out-projection: attention is done, so the wide 'sc' psum
            # tiles are free -- both 512-halves of a P stripe land in one
            # tile, one copy (alternating DVE/ACT), one DMA; PE stays warm
            # on a dense mm stream
            Copy = mybir.ActivationFunctionType.Copy
            for st2 in range(QB // 128):
                st = 1 * (QB // 128) + st2
                op = ps_pool.tile([128, QB], f32, tag="sc", bufs=2)
                for nch in range(2):
                    nc.tensor.matmul(
                        op[:, nch * 512 : (nch + 1) * 512],
                        aT11[:, st2 * 128 : (st2 + 1) * 128],
                        ow_sb[:, nch * 512 : (nch + 1) * 512],
                        start=True,
                        stop=True,
                        skip_group_check=True,
                    )
                psb = osb.tile([128, E], f32, tag="ptile", bufs=3)
                if st2 % 2 == 0:
                    nc.scalar.activation(
                        psb[:, 0:512], op[:, 0:512], Copy
                    )
                    nc.vector.tensor_copy(psb[:, 512:1024], op[:, 512:1024])
                else:
                    nc.vector.tensor_copy(psb[:, 0:512], op[:, 0:512])
                    nc.scalar.activation(
                        psb[:, 512:1024], op[:, 512:1024], Copy
                    )
                nc.sync.dma_start(
                    P_dram[1][st * 128 : (st + 1) * 128, :], psb[:]
                )
            if with_cc:
                # the real collective reads all of P(1), so it must be
                # emitted after every P write (program order = dependency)
                reduce_scatter(1)

    nc.compile()
    return nc


def _host_inputs(query, Wq, bq, Wk, bk, Wv, bv, Wo, bo):
    """Per-core input maps (all fp32, C-contiguous)."""
    scaling = HD ** (-0.5)

    invf = 1.0 / (
        10000.0 ** (np.arange(0, HD, 2, dtype=np.float32) / np.float32(HD))
    )
    t = np.arange(S, dtype=np.float32)
    fr = np.outer(t, invf).astype(np.float32)  # [S, 32]
    emb = np.concatenate([fr, fr], axis=1)  # [S, HD]
    cosT = np.cos(emb).T.astype(np.float32)  # [HD, S]
    sinT = np.sin(emb).T.astype(np.float32)
    sign = np.where(np.arange(HD) < HD // 2, -1.0, 1.0).astype(np.float32)[:, None]
    cos_t = np.ascontiguousarray(np.tile(cosT, (HPC, 1)), dtype=np.float32)
    sin_t = np.ascontiguousarray(np.tile(sinT * sign, (HPC, 1)), dtype=np.float32)

    query = np.ascontiguousarray(query, dtype=np.float32)
    in_maps = []
    for c in range(NCORES):
        sl = slice(c * EL, (c + 1) * EL)
        in_maps.append(
            {
                "query": query,
                "q_w": np.ascontiguousarray((Wq[sl, :] * scaling).T, dtype=np.float32),
                "k_w": np.ascontiguousarray(Wk[sl, :].T, dtype=np.float32),
                "v_w": np.ascontiguousarray(Wv[sl, :].T, dtype=np.float32),
                "o_w": np.ascontiguousarray(Wo[:, sl].T, dtype=np.float32),
                "bq_s": np.ascontiguousarray(
                    (bq[sl] * scaling).reshape(EL, 1), dtype=np.float32
                ),
                "bk_s": np.ascontiguousarray(bk[sl].reshape(EL, 1), dtype=np.float32),
                "bv_s": np.ascontiguousarray(bv[sl].reshape(EL, 1), dtype=np.float32),
                "cos_t": cos_t,
                "sin_t": sin_t,
            }
        )
    return in_maps


def kernel(query, Wq, bq, Wk, bk, Wv, bv, Wo, bo):
    global LAST_RESULT
    from concourse.bass_utils import run_bass_kernel_spmd

    if "nc" not in _CACHE:
        _CACHE["nc"] = _build_program()
    nc = _CACHE["nc"]

    in_maps = _host_inputs(
        np.asarray(query),
        np.asarray(Wq),
        np.asarray(bq),
        np.asarray(Wk),
        np.asarray(bk),
        np.asarray(Wv),
        np.asarray(bv),
        np.asarray(Wo),
        np.asarray(bo),
    )
    res = run_bass_kernel_spmd(nc, in_maps, core_ids=list(range(NCORES)))
    LAST_RESULT = res
    shards = [
        res.results[c]["out"].reshape(B, S // NCORES, E).transpose(1, 0, 2)
        for c in range(NCORES)
    ]
    full = np.concatenate(shards, axis=0)
    # output bias is added once on the host (not per-core: the ReduceScatter
    # sums partial projections from all 8 cores)
    return full + np.asarray(bo, dtype=np.float32).reshape(1, 1, E)
